# revision 1
# baseline (speedup 1.0000x reference)
"""TRN2 Bass kernel for nn_Attention (cross-attention, Tq=2, Tk=5, B=16384, D=512).

Math reformulation (exact):
    logits = h . k~,  k~ = e @ W_qk,  W_qk = Wk @ Wq^T
    att = softmax(logits)
    out = h@Wd1 + ctx@W_vd,   ctx = att @ e,   W_vd = Wv @ Wd2
This removes the q and v projections entirely.

Sharding: pure data parallel over batch, 2048 per core x 8 cores.
Host marshals inputs/outputs to batch-major [B, T, D] for contiguous DMA.
On-chip compute in fp16 (PSUM accumulation fp32); final out fp32.
Main loop is software-pipelined (front: loads/transposes/k~; back: attention/out)
with a lag of 2 batch tiles so PE and DVE streams interleave across tiles.
"""

import contextlib

import numpy as np

import concourse.bass as bass
import concourse.mybir as mybir
import concourse.tile as tile
from concourse import bacc
from concourse.bass_utils import run_bass_kernel_spmd
from concourse.masks import make_identity

F32 = mybir.dt.float32
F16 = mybir.dt.float16
MUL = mybir.AluOpType.mult
ADD = mybir.AluOpType.add
BYP = mybir.AluOpType.bypass

TQ, TK, B, D = 2, 5, 16384, 512
NCORES = 8
BL = B // NCORES          # 2048 batch per core
P = 128                   # partition tile
NT = BL // P              # 16 batch tiles per core
DC = D // P               # 4 contraction chunks
LAG = 3                   # software-pipeline depth (front of t  ||  back of t-LAG)

_CACHED = {}


def build(reps=1, skip=()):
    nc = bacc.Bacc("TRN2", target_bir_lowering=False, debug=False)

    h_d = nc.dram_tensor("h", [BL, TQ, D], F16, kind="ExternalInput")
    e_d = nc.dram_tensor("enc", [BL, TK, D], F16, kind="ExternalInput")
    ht_d = nc.dram_tensor("hT", [NT, P, DC, TQ, P], F16, kind="ExternalInput")
    et_d = nc.dram_tensor("eT", [NT, P, DC, TK, P], F16, kind="ExternalInput")
    wq_d = nc.dram_tensor("Wq", [D, D], F32, kind="ExternalInput")
    wk_d = nc.dram_tensor("Wk", [D, D], F32, kind="ExternalInput")
    wv_d = nc.dram_tensor("Wv", [D, D], F32, kind="ExternalInput")
    wd_d = nc.dram_tensor("Wdown", [2 * D, D], F32, kind="ExternalInput")
    o_d = nc.dram_tensor("out", [BL, TQ, D], F32, kind="ExternalOutput")

    h_r = h_d.ap()
    e_r = e_d.ap()
    o_r = o_d.ap()

    with tile.TileContext(nc) as tc:
        with (
            tc.tile_pool(name="wgt", bufs=1) as wgt,
            tc.tile_pool(name="pre", bufs=1) as pre,
            tc.tile_pool(name="io", bufs=LAG + 2) as io,
            tc.tile_pool(name="work", bufs=LAG + 2) as work,
            tc.tile_pool(name="bwork", bufs=2) as bwork,
            tc.tile_pool(name="small", bufs=3) as small,
            tc.tile_pool(name="ps", bufs=2, space="PSUM") as ps,       # "pt": [P,8,P] 2bk x2
            tc.tile_pool(name="psk", bufs=2, space="PSUM") as psk,     # "pk": [P,512] 1bk x2
            tc.tile_pool(name="psb", bufs=2, space="PSUM") as psb,     # "po": 1bk x2
        ):
            ident = wgt.tile([P, P], F16)
            make_identity(nc, ident)

            # ---- load weights (cast fp32 -> fp16 during DMA) ----
            wq16 = pre.tile([P, DC, D], F16, tag="wq16")
            wk16 = pre.tile([P, DC, D], F16, tag="wk16")
            wv16 = pre.tile([P, DC, D], F16, tag="wv16")
            wd1 = wgt.tile([P, DC, D], F16, tag="wd1")
            wd2 = wgt.tile([P, DC, D], F16, tag="wd2")
            nc.gpsimd.dma_start(out=wq16, in_=wq_d.ap().rearrange("(c p) n -> p c n", p=P))
            nc.gpsimd.dma_start(out=wk16, in_=wk_d.ap().rearrange("(c p) n -> p c n", p=P))
            nc.gpsimd.dma_start(out=wv16, in_=wv_d.ap().rearrange("(c p) n -> p c n", p=P))
            nc.gpsimd.dma_start(out=wd1, in_=wd_d.ap()[:D].rearrange("(c p) n -> p c n", p=P))
            nc.gpsimd.dma_start(out=wd2, in_=wd_d.ap()[D:].rearrange("(c p) n -> p c n", p=P))

            # ---- transpose Wq, Wk, Wv via identity-matmul (2 waves of 8 blocks) ----
            def transpose_weight(w16, name):
                wT = pre.tile([P, DC, D], F16, tag=name, name=name)
                for w in range(2):
                    pt = ps.tile([P, 8, P], F32, tag="pt", name=f"pt_{name}{w}")
                    for gg in range(2):
                        g = w * 2 + gg
                        for a in range(DC):
                            nc.tensor.matmul(
                                pt[:, gg * 4 + a, :], w16[:, a, g * P:(g + 1) * P],
                                ident, start=True, stop=True)
                    nc.scalar.copy(
                        wT[:, w * 2:w * 2 + 2, :],
                        pt.rearrange("p (g a) b -> p g (a b)", g=2))
                return wT

            wqT = transpose_weight(wq16, "wqT")
            wkT = transpose_weight(wk16, "wkT")
            wvT = transpose_weight(wv16, "wvT")

            # ---- W_qk = Wk @ Wq^T ;  W_vd = Wv @ Wd2 ----
            wqk = wgt.tile([P, DC, D], F16, tag="wqk")
            wvd = wgt.tile([P, DC, D], F16, tag="wvd")
            for nm, (lhsT, rhs, dst) in {
                "q": (wkT, wqT, wqk), "v": (wvT, wd2, wvd)
            }.items():
                for ach in range(DC):
                    acc = psk.tile([P, D], F32, tag="pk", name=f"pk_{nm}{ach}")
                    for g in range(DC):
                        nc.tensor.matmul(
                            acc, lhsT[:, g, ach * P:(ach + 1) * P],
                            rhs[:, g, :], start=(g == 0), stop=(g == DC - 1))
                    nc.scalar.copy(dst[:, ach, :], acc)

            # ---- preload variant (for DMA-ablation benchmarking) ----
            pre_hn, pre_en = [], []
            if "dma" in skip:
                for t in range(NT):
                    phn = pre.tile([P, TQ, D], F16, tag=f"phn{t}", name=f"phn{t}")
                    pen = pre.tile([P, TK, D], F16, tag=f"pen{t}", name=f"pen{t}")
                    nc.gpsimd.dma_start(out=phn, in_=h_r[t * P:(t + 1) * P])
                    nc.gpsimd.dma_start(out=pen, in_=e_r[t * P:(t + 1) * P])
                    pre_hn.append(phn)
                    pre_en.append(pen)

            # ================= software-pipelined main loop =================
            def emit_front(t):
                bsl = slice(t * P, (t + 1) * P)
                if "dma" in skip:
                    hn, en = pre_hn[t], pre_en[t]
                else:
                    hn = io.tile([P, TQ, D], F16, tag="hn", name=f"hn{t}")
                    en = io.tile([P, TK, D], F16, tag="en", name=f"en{t}")
                    nc.sync.dma_start(out=hn, in_=h_r[bsl])
                    nc.sync.dma_start(out=en, in_=e_r[bsl])

                hT = work.tile([P, DC, TQ, P], F16, tag="hT", name=f"hT{t}")
                nc.sync.dma_start(out=hT, in_=ht_d.ap()[t])
                eT = work.tile([P, DC, TK, P], F16, tag="eT", name=f"eT{t}")
                nc.sync.dma_start(out=eT, in_=et_d.ap()[t])

                kn = work.tile([P, TK, D], F16, tag="kn", name=f"kn{t}")
                for j in range(TK):
                    acc = psk.tile([P, D], F32, tag="pk", name=f"pkk{t}_{j}")
                    for c in range(DC):
                        nc.tensor.matmul(
                            acc, eT[:, c, j, :], wqk[:, c, :],
                            start=(c == 0), stop=(c == DC - 1))
                    nc.scalar.copy(kn[:, j, :], acc)

                return dict(t=t, hn=hn, en=en, hT=hT, kn=kn)

            def emit_back(st):
                t, hn, en, hT, kn = st["t"], st["hn"], st["en"], st["hT"], st["kn"]
                bsl = slice(t * P, (t + 1) * P)

                if "attn" in skip:
                    ctx = bwork.tile([P, TQ, D], F16, tag="ctx", name=f"ctx{t}")
                    nc.vector.tensor_copy(ctx, kn[:, 0:2, :])
                else:
                    logits = small.tile([P, TQ, TK], F32, tag="logits", name=f"lg{t}")
                    pdump = small.tile([P, 1], F16, tag="pdump", name=f"pd{t}")
                    for i in range(TQ):
                        for j in range(TK):
                            nc.vector.scalar_tensor_tensor(
                                out=pdump.broadcast_to([P, D]),
                                in0=hn[:, i, :], scalar=1.0, in1=kn[:, j, :],
                                op0=BYP, op1=MUL,
                                accum_out=logits[:, i, j:j + 1])

                    nmx = small.tile([P, TQ], F32, tag="nmx", name=f"nm{t}")
                    pr = small.tile([P, TQ, TK], F32, tag="pr", name=f"pr{t}")
                    sm = small.tile([P, TQ], F32, tag="sm", name=f"sm{t}")
                    rs = small.tile([P, TQ], F32, tag="rs", name=f"rs{t}")
                    attw = small.tile([P, TQ, TK], F32, tag="attw", name=f"at{t}")
                    nc.vector.tensor_reduce(
                        out=nmx, in_=logits, axis=mybir.AxisListType.X,
                        op=mybir.AluOpType.max, negate=True)
                    for i in range(TQ):
                        nc.scalar.activation(
                            out=pr[:, i, :], in_=logits[:, i, :],
                            func=mybir.ActivationFunctionType.Exp,
                            bias=nmx[:, i:i + 1],
                            accum_out=sm[:, i:i + 1])
                    nc.vector.reciprocal(rs, sm)
                    for i in range(TQ):
                        nc.vector.tensor_scalar_mul(attw[:, i, :], pr[:, i, :], rs[:, i:i + 1])

                    ctx = bwork.tile([P, TQ, D], F16, tag="ctx", name=f"ctx{t}")
                    for i in range(TQ):
                        nc.vector.tensor_scalar_mul(ctx[:, i, :], en[:, 0, :], attw[:, i, 0:1])
                        for j in range(1, TK):
                            nc.vector.scalar_tensor_tensor(
                                out=ctx[:, i, :], in0=en[:, j, :],
                                scalar=attw[:, i, j:j + 1], in1=ctx[:, i, :],
                                op0=MUL, op1=ADD)

                cT = bwork.tile([P, DC, TQ, P], F16, tag="cT", name=f"cT{t}")
                ptc = ps.tile([P, 8, P], F32, tag="pt", name=f"ptc{t}")
                for i in range(TQ):
                    for c in range(DC):
                        nc.tensor.matmul(
                            ptc[:, i * DC + c, :], ctx[:, i, c * P:(c + 1) * P],
                            ident, start=True, stop=True)
                nc.scalar.copy(cT, ptc.rearrange("p (i c) b -> p c i b", i=TQ))

                ob = io.tile([P, TQ, D], F32, tag="ob", name=f"ob{t}")
                for i in range(TQ):
                    po = psb.tile([P, D], F32, tag="po", name=f"po{t}_{i}")
                    for c in range(DC):
                        nc.tensor.matmul(po, hT[:, c, i, :], wd1[:, c, :],
                                         start=(c == 0), stop=False)
                    for c in range(DC):
                        nc.tensor.matmul(po, cT[:, c, i, :], wvd[:, c, :],
                                         start=False, stop=(c == DC - 1))
                    nc.scalar.copy(ob[:, i, :], po)
                for i in range(TQ):
                    nc.sync.dma_start(out=o_r[bsl][:, i, :], in_=ob[:, i, :])

            loop_cm = tc.For_i(0, reps, 1) if reps > 1 else contextlib.nullcontext()
            with loop_cm:
                pending = {}
                for tt in range(NT + LAG):
                    if tt < NT:
                        pending[tt] = emit_front(tt)
                    if tt >= LAG:
                        emit_back(pending.pop(tt - LAG))

    nc.compile()
    return nc


def kernel(h, enc_out, Wq, Wk, Wv, Wdown, _trace=False):
    h = np.ascontiguousarray(h, dtype=np.float32)
    enc_out = np.ascontiguousarray(enc_out, dtype=np.float32)
    Wq = np.ascontiguousarray(Wq, dtype=np.float32)
    Wk = np.ascontiguousarray(Wk, dtype=np.float32)
    Wv = np.ascontiguousarray(Wv, dtype=np.float32)
    Wdown = np.ascontiguousarray(Wdown, dtype=np.float32)

    if "nc" not in _CACHED:
        _CACHED["nc"] = build()
    nc = _CACHED["nc"]

    h16 = h.astype(np.float16)
    e16 = enc_out.astype(np.float16)
    h_bm = np.ascontiguousarray(h16.transpose(1, 0, 2))        # [B, TQ, D]
    e_bm = np.ascontiguousarray(e16.transpose(1, 0, 2))        # [B, TK, D]
    # block-transposed tiles: [core][t][p(d%128)][c][i][b]
    hT_bm = np.ascontiguousarray(
        h16.reshape(TQ, NCORES, NT, P, DC, P).transpose(1, 2, 5, 4, 0, 3))
    eT_bm = np.ascontiguousarray(
        e16.reshape(TK, NCORES, NT, P, DC, P).transpose(1, 2, 5, 4, 0, 3))
    in_maps = []
    for c in range(NCORES):
        sl = slice(c * BL, (c + 1) * BL)
        in_maps.append({
            "h": h_bm[sl],
            "enc": e_bm[sl],
            "hT": hT_bm[c],
            "eT": eT_bm[c],
            "Wq": Wq, "Wk": Wk, "Wv": Wv, "Wdown": Wdown,
        })

    res = run_bass_kernel_spmd(nc, in_maps, list(range(NCORES)), trace=_trace)
    out_bm = np.concatenate([r["out"] for r in res.results], axis=0)  # [B, TQ, D]
    out = np.ascontiguousarray(out_bm.transpose(1, 0, 2))
    if _trace:
        kernel.last_result = res
    return out.astype(np.float32)



# revision 3
# speedup vs baseline: 1.4760x; 1.4760x over previous
"""TRN2 Bass kernel for nn_Attention (cross-attention, Tq=2, Tk=5, B=16384, D=512).

Math reformulation (exact):
    q~ = h @ W_A,        W_A  = Wq @ Wk^T          (host-precomputed, tiny)
    logits[b,i,j] = q~[b,i,:] . e[b,j,:]           (DVE dots, fp32 accum)
    w = softmax(logits)  (normalization folded into the diag weights)
    ctx[b,i,:] = sum_j w[b,i,j] * e[b,j,:]         (PE: diag(w) matmuls, PSUM accum)
    out = h @ Wd1 + ctx @ W_vd,  W_vd = Wv @ Wd2   (host-precomputed, tiny)

Per-batch weighted sums run on the PE via diagonal stationary matrices:
    matmul(psum, lhsT=diag(w_ij), rhs=e_j)  accumulates w_ij[b]*e[b,j,:] per lane.
diag(w_ij) is built with one 4x-mode tensor_scalar op on a fp16 identity
(identity * exp_ij * recip_sum_i), so softmax normalization costs nothing extra.

Sharding: pure data parallel over batch, 2048 per core x 8 cores.
Host marshals e to batch-major [B, Tk, D] fp16 and h to block-transposed
lhsT layout [NT, P(d), DC, Tq, P(b)] fp16. Output fp16, upcast on host.
Main loop software-pipelined with LAG tiles between q~ production (front)
and the attention/output stages (back).
"""

import contextlib

import numpy as np

import concourse.bass as bass
import concourse.mybir as mybir
import concourse.tile as tile
from concourse import bacc
from concourse.bass_utils import run_bass_kernel_spmd
from concourse.masks import make_identity

F32 = mybir.dt.float32
F16 = mybir.dt.float16
MUL = mybir.AluOpType.mult
ADD = mybir.AluOpType.add
BYP = mybir.AluOpType.bypass

TQ, TK, B, D = 2, 5, 16384, 512
NCORES = 8
BL = B // NCORES          # 2048 batch per core
P = 128                   # partition tile
NT = BL // P              # 16 batch tiles per core
DC = D // P               # 4 contraction chunks
LAG = 2                   # software-pipeline depth (front of t || back of t-LAG)

_CACHED = {}


def build():
    nc = bacc.Bacc("TRN2", target_bir_lowering=False, debug=False)

    e_d = nc.dram_tensor("enc", [BL, TK, D], F16, kind="ExternalInput")
    ht_d = nc.dram_tensor("hT", [NT, P, DC, TQ, P], F16, kind="ExternalInput")
    wqk_d = nc.dram_tensor("Wqk", [P, DC, D], F16, kind="ExternalInput")
    wd1_d = nc.dram_tensor("Wd1", [P, DC, D], F16, kind="ExternalInput")
    wvd_d = nc.dram_tensor("Wvd", [P, DC, D], F16, kind="ExternalInput")
    o_d = nc.dram_tensor("out", [BL, TQ, D], F16, kind="ExternalOutput")

    e_r = e_d.ap()
    o_r = o_d.ap()

    with tile.TileContext(nc) as tc:
        with (
            tc.tile_pool(name="wgt", bufs=1) as wgt,
            tc.tile_pool(name="io", bufs=LAG + 2) as io,
            tc.tile_pool(name="qp", bufs=LAG + 2) as qp,
            tc.tile_pool(name="work", bufs=2) as work,
            tc.tile_pool(name="small", bufs=3) as small,
            tc.tile_pool(name="obp", bufs=2) as obp,
            tc.tile_pool(name="psq", bufs=1, space="PSUM") as psq,   # [P,TQ,D] 2bk
            tc.tile_pool(name="psc", bufs=1, space="PSUM") as psc,   # [P,TQ,D] 2bk
            tc.tile_pool(name="pst", bufs=1, space="PSUM") as pst,   # [P,8,P]  2bk
            tc.tile_pool(name="pso", bufs=2, space="PSUM") as pso,   # [P,D] 1bk x2
        ):
            ident = wgt.tile([P, P], F16)
            make_identity(nc, ident)

            wqk = wgt.tile([P, DC, D], F16, tag="wqk")
            wd1 = wgt.tile([P, DC, D], F16, tag="wd1")
            wvd = wgt.tile([P, DC, D], F16, tag="wvd")
            nc.gpsimd.dma_start(out=wqk, in_=wqk_d.ap())
            nc.gpsimd.dma_start(out=wd1, in_=wd1_d.ap())
            nc.gpsimd.dma_start(out=wvd, in_=wvd_d.ap())

            # ================= software-pipelined main loop =================
            def emit_front(t):
                bsl = slice(t * P, (t + 1) * P)
                en = io.tile([P, TK, D], F16, tag="en", name=f"en{t}")
                nc.sync.dma_start(out=en, in_=e_r[bsl])
                hT = io.tile([P, DC, TQ, P], F16, tag="hT", name=f"hT{t}")
                nc.sync.dma_start(out=hT, in_=ht_d.ap()[t])

                # q~ = h @ W_A   [P, TQ, D]
                pq = psq.tile([P, TQ, D], F32, tag="pq", name=f"pq{t}")
                for i in range(TQ):
                    for c in range(DC):
                        nc.tensor.matmul(
                            pq[:, i, :], hT[:, c, i, :], wqk[:, c, :],
                            start=(c == 0), stop=(c == DC - 1))
                qn = qp.tile([P, TQ, D], F16, tag="qn", name=f"qn{t}")
                nc.scalar.copy(qn, pq)

                return dict(t=t, en=en, hT=hT, qn=qn)

            def emit_back(st):
                t, en, hT, qn = st["t"], st["en"], st["hT"], st["qn"]
                bsl = slice(t * P, (t + 1) * P)

                # logits[b,i,j] = q~_i . e_j  (DVE 1x dots, fp32 accumulator)
                lg = small.tile([P, TQ, TK], F32, tag="lg", name=f"lg{t}")
                pdump = small.tile([P, 1], F16, tag="pdump", name=f"pd{t}")
                for i in range(TQ):
                    for j in range(TK):
                        nc.vector.scalar_tensor_tensor(
                            out=pdump.broadcast_to([P, D]),
                            in0=qn[:, i, :], scalar=1.0, in1=en[:, j, :],
                            op0=BYP, op1=MUL,
                            accum_out=lg[:, i, j:j + 1])

                nmx = small.tile([P, TQ], F32, tag="nmx", name=f"nm{t}")
                nc.vector.tensor_reduce(
                    out=nmx, in_=lg, axis=mybir.AxisListType.X,
                    op=mybir.AluOpType.max, negate=True)
                pr = small.tile([P, TQ, TK], F32, tag="pr", name=f"pr{t}")
                sm = small.tile([P, TQ], F32, tag="sm", name=f"sm{t}")
                rs = small.tile([P, TQ], F32, tag="rs", name=f"rs{t}")
                for i in range(TQ):
                    nc.scalar.activation(
                        out=pr[:, i, :], in_=lg[:, i, :],
                        func=mybir.ActivationFunctionType.Exp,
                        bias=nmx[:, i:i + 1],
                        accum_out=sm[:, i:i + 1])
                nc.vector.reciprocal(rs, sm)

                # diag(w_ij) = ident * exp_ij * (1/sum_i)   (4x-mode DVE)
                dg = work.tile([P, TQ, TK, P], F16, tag="dg", name=f"dg{t}")
                for i in range(TQ):
                    for j in range(TK):
                        nc.vector.tensor_scalar(
                            out=dg[:, i, j, :], in0=ident,
                            scalar1=pr[:, i, j:j + 1], scalar2=rs[:, i:i + 1],
                            op0=MUL, op1=MUL)

                # ctx_i = sum_j diag(w_ij) @ e_j   (PE, PSUM accumulation)
                pc = psc.tile([P, TQ, D], F32, tag="pc", name=f"pc{t}")
                for i in range(TQ):
                    for j in range(TK):
                        nc.tensor.matmul(
                            pc[:, i, :], dg[:, i, j, :], en[:, j, :],
                            start=(j == 0), stop=(j == TK - 1))
                cx = work.tile([P, TQ, D], F16, tag="cx", name=f"cx{t}")
                nc.scalar.copy(cx, pc)

                # transpose ctx -> cT [P(d), DC, TQ, P(b)]
                pt = pst.tile([P, TQ * DC, P], F16, tag="pt", name=f"pt{t}")
                for i in range(TQ):
                    for c in range(DC):
                        nc.tensor.transpose(
                            pt[:, i * DC + c, :],
                            cx[:, i, c * P:(c + 1) * P], ident)
                cT = work.tile([P, DC, TQ, P], F16, tag="cT", name=f"cT{t}")
                nc.scalar.copy(cT, pt.rearrange("p (i c) b -> p c i b", i=TQ))

                # out_i = h_i @ Wd1 + ctx_i @ Wvd
                ob = obp.tile([P, TQ, D], F16, tag="ob", name=f"ob{t}")
                for i in range(TQ):
                    po = pso.tile([P, D], F32, tag="po", name=f"po{t}_{i}")
                    for c in range(DC):
                        nc.tensor.matmul(po, hT[:, c, i, :], wd1[:, c, :],
                                         start=(c == 0), stop=False)
                    for c in range(DC):
                        nc.tensor.matmul(po, cT[:, c, i, :], wvd[:, c, :],
                                         start=False, stop=(c == DC - 1))
                    nc.scalar.copy(ob[:, i, :], po)
                nc.sync.dma_start(out=o_r[bsl], in_=ob)

            pending = {}
            for tt in range(NT + LAG):
                if tt < NT:
                    pending[tt] = emit_front(tt)
                if tt >= LAG:
                    emit_back(pending.pop(tt - LAG))

    nc.compile()
    return nc


def _pack_w(w):
    # [D, D] fp32 -> [P, DC, D] fp16 with contraction dim on partitions
    return np.ascontiguousarray(
        w.reshape(DC, P, D).transpose(1, 0, 2).astype(np.float16))


def kernel(h, enc_out, Wq, Wk, Wv, Wdown, _trace=False):
    h = np.ascontiguousarray(h, dtype=np.float32)
    enc_out = np.ascontiguousarray(enc_out, dtype=np.float32)
    Wq = np.ascontiguousarray(Wq, dtype=np.float32)
    Wk = np.ascontiguousarray(Wk, dtype=np.float32)
    Wv = np.ascontiguousarray(Wv, dtype=np.float32)
    Wdown = np.ascontiguousarray(Wdown, dtype=np.float32)

    if "nc" not in _CACHED:
        _CACHED["nc"] = build()
    nc = _CACHED["nc"]

    w_a = _pack_w(Wq @ Wk.T)
    w_d1 = _pack_w(Wdown[:D])
    w_vd = _pack_w(Wv @ Wdown[D:])

    h16 = h.astype(np.float16)
    e16 = enc_out.astype(np.float16)
    e_bm = np.ascontiguousarray(e16.transpose(1, 0, 2))        # [B, TK, D]
    # block-transposed lhsT tiles: [core][t][p(d%128)][c][i][p(b%128)]
    hT_bm = np.ascontiguousarray(
        h16.reshape(TQ, NCORES, NT, P, DC, P).transpose(1, 2, 5, 4, 0, 3))
    in_maps = []
    for c in range(NCORES):
        sl = slice(c * BL, (c + 1) * BL)
        in_maps.append({
            "enc": e_bm[sl],
            "hT": hT_bm[c],
            "Wqk": w_a, "Wd1": w_d1, "Wvd": w_vd,
        })

    res = run_bass_kernel_spmd(nc, in_maps, list(range(NCORES)), trace=_trace)
    out_bm = np.concatenate([r["out"] for r in res.results], axis=0)  # [B, TQ, D]
    out = np.ascontiguousarray(out_bm.transpose(1, 0, 2))
    if _trace:
        kernel.last_result = res
    return out.astype(np.float32)


# revision 4
# speedup vs baseline: 1.6051x; 1.0875x over previous
"""TRN2 Bass kernel for nn_Attention (cross-attention, Tq=2, Tk=5, B=16384, D=512).

Math reformulation (exact):
    q~ = h @ W_A,        W_A  = Wq @ Wk^T          (host-precomputed, tiny)
    logits[b,i,j] = q~[b,i,:] . e[b,j,:]           (DVE dots, fp32 accum)
    ex = exp(logits - max)                          (Act)
    ctxu[b,i,:] = sum_j ex[b,i,j] * e[b,j,:]       (PE: diag(ex) matmuls, PSUM accum)
    ctx = ctxu / sum_j ex                           (folded into Act PSUM->SBUF copy)
    out = h @ Wd1 + ctx @ W_vd,  W_vd = Wv @ Wd2   (host-precomputed, tiny)

Per-batch weighted sums run on the PE via diagonal stationary matrices:
    matmul(psum, lhsT=diag(ex_ij), rhs=e_j)  accumulates ex_ij[b]*e[b,j,:] per lane.
diag(ex_ij) is a single-scalar 4x-mode tensor_scalar op on a fp16 identity.
Softmax normalization rides the Act-engine copy (per-partition scale = 1/sum).

Sharding: pure data parallel over batch, 2048 per core x 8 cores.
Host marshals e to batch-major [B, Tk, D] fp16 and h to block-transposed
lhsT layout [NT, P(d), DC, Tq, P(b)] fp16. Output fp16, upcast on host.
Main loop is a 3-stage software pipeline (A: loads+q~ | B: dots+max+exp |
C: recip+diag+ctx+transpose+out) so the DVE never stalls on Act's EXP.
"""

import numpy as np

import concourse.bass as bass
import concourse.mybir as mybir
import concourse.tile as tile
from concourse import bacc
from concourse.bass_utils import run_bass_kernel_spmd
from concourse.masks import make_identity

F32 = mybir.dt.float32
F16 = mybir.dt.float16
MUL = mybir.AluOpType.mult
ADD = mybir.AluOpType.add
BYP = mybir.AluOpType.bypass

TQ, TK, B, D = 2, 5, 16384, 512
NCORES = 8
BL = B // NCORES          # 2048 batch per core
P = 128                   # partition tile
NT = BL // P              # 16 batch tiles per core
DC = D // P               # 4 contraction chunks

_CACHED = {}


def build():
    nc = bacc.Bacc("TRN2", target_bir_lowering=False, debug=False)

    e_d = nc.dram_tensor("enc", [BL, TK, D], F16, kind="ExternalInput")
    ht_d = nc.dram_tensor("hT", [NT, P, DC, TQ, P], F16, kind="ExternalInput")
    wqk_d = nc.dram_tensor("Wqk", [P, DC, D], F16, kind="ExternalInput")
    wd1_d = nc.dram_tensor("Wd1", [P, DC, D], F16, kind="ExternalInput")
    wvd_d = nc.dram_tensor("Wvd", [P, DC, D], F16, kind="ExternalInput")
    o_d = nc.dram_tensor("out", [BL, TQ, D], F16, kind="ExternalOutput")

    e_r = e_d.ap()
    o_r = o_d.ap()

    with tile.TileContext(nc) as tc:
        with (
            tc.tile_pool(name="wgt", bufs=1) as wgt,
            tc.tile_pool(name="io", bufs=4) as io,
            tc.tile_pool(name="qp", bufs=4) as qp,
            tc.tile_pool(name="work", bufs=2) as work,
            tc.tile_pool(name="small", bufs=4) as small,
            tc.tile_pool(name="scr", bufs=2) as scr,
            tc.tile_pool(name="obp", bufs=2) as obp,
            tc.tile_pool(name="psq", bufs=1, space="PSUM") as psq,   # [P,TQ,D]f32 2bk
            tc.tile_pool(name="psc", bufs=1, space="PSUM") as psc,   # [P,TQ,D]f32 2bk
            tc.tile_pool(name="pst", bufs=2, space="PSUM") as pst,   # [P,8,P]f16 1bk x2
            tc.tile_pool(name="pso", bufs=2, space="PSUM") as pso,   # [P,D]f32 1bk x2
        ):
            ident = wgt.tile([P, P], F16)
            make_identity(nc, ident)

            wqk = wgt.tile([P, DC, D], F16, tag="wqk")
            wd1 = wgt.tile([P, DC, D], F16, tag="wd1")
            wvd = wgt.tile([P, DC, D], F16, tag="wvd")
            nc.gpsimd.dma_start(out=wqk, in_=wqk_d.ap())
            nc.gpsimd.dma_start(out=wd1, in_=wd1_d.ap())
            nc.gpsimd.dma_start(out=wvd, in_=wvd_d.ap())

            # ================= 3-stage software-pipelined loop =================
            def stage_a(t):
                bsl = slice(t * P, (t + 1) * P)
                en = io.tile([P, TK, D], F16, tag="en", name=f"en{t}")
                nc.sync.dma_start(out=en, in_=e_r[bsl])
                hT = io.tile([P, DC, TQ, P], F16, tag="hT", name=f"hT{t}")
                nc.sync.dma_start(out=hT, in_=ht_d.ap()[t])

                # q~ = h @ W_A   [P, TQ, D]
                pq = psq.tile([P, TQ, D], F32, tag="pq", name=f"pq{t}")
                for i in range(TQ):
                    for c in range(DC):
                        nc.tensor.matmul(
                            pq[:, i, :], hT[:, c, i, :], wqk[:, c, :],
                            start=(c == 0), stop=(c == DC - 1))
                qn = qp.tile([P, TQ, D], F16, tag="qn", name=f"qn{t}")
                nc.scalar.copy(qn, pq)

                return dict(t=t, en=en, hT=hT, qn=qn)

            def stage_b(st):
                t, en, qn = st["t"], st["en"], st["qn"]

                # logits[b,i,j] = q~_i . e_j  (DVE 1x dots, fp32 accumulator)
                lg = small.tile([P, TQ, TK], F32, tag="lg", name=f"lg{t}")
                dump = scr.tile([P, D], F16, tag="dump", name=f"du{t}")
                for i in range(TQ):
                    for j in range(TK):
                        nc.vector.scalar_tensor_tensor(
                            out=dump,
                            in0=qn[:, i, :], scalar=1.0, in1=en[:, j, :],
                            op0=BYP, op1=MUL,
                            accum_out=lg[:, i, j:j + 1])

                nmx = small.tile([P, TQ], F32, tag="nmx", name=f"nm{t}")
                nc.vector.tensor_reduce(
                    out=nmx, in_=lg, axis=mybir.AxisListType.X,
                    op=mybir.AluOpType.max, negate=True)
                pr = small.tile([P, TQ, TK], F32, tag="pr", name=f"pr{t}")
                sm = small.tile([P, TQ], F32, tag="sm", name=f"sm{t}")
                for i in range(TQ):
                    nc.scalar.activation(
                        out=pr[:, i, :], in_=lg[:, i, :],
                        func=mybir.ActivationFunctionType.Exp,
                        bias=nmx[:, i:i + 1],
                        accum_out=sm[:, i:i + 1])
                st.update(pr=pr, sm=sm)
                return st

            def stage_c(st):
                t, en, hT, pr, sm = st["t"], st["en"], st["hT"], st["pr"], st["sm"]
                bsl = slice(t * P, (t + 1) * P)

                rs = small.tile([P, TQ], F32, tag="rs", name=f"rs{t}")
                nc.vector.reciprocal(rs, sm)

                # diag(ex_ij) = ident * ex_ij   (single-scalar 4x-mode DVE)
                dg = work.tile([P, TQ, TK, P], F16, tag="dg", name=f"dg{t}")
                for i in range(TQ):
                    for j in range(TK):
                        nc.vector.tensor_scalar_mul(
                            dg[:, i, j, :], ident, pr[:, i, j:j + 1])

                # ctxu_i = sum_j diag(ex_ij) @ e_j   (PE, PSUM accumulation)
                pc = psc.tile([P, TQ, D], F32, tag="pc", name=f"pc{t}")
                for i in range(TQ):
                    for j in range(TK):
                        nc.tensor.matmul(
                            pc[:, i, :], dg[:, i, j, :], en[:, j, :],
                            start=(j == 0), stop=(j == TK - 1))
                # normalize during PSUM->SBUF copy: ctx_i = ctxu_i * (1/sum_i)
                cx = work.tile([P, TQ, D], F16, tag="cx", name=f"cx{t}")
                for i in range(TQ):
                    nc.scalar.mul(cx[:, i, :], pc[:, i, :], rs[:, i:i + 1])

                # transpose ctx -> cT [P(d), DC, TQ, P(b)]
                pt = pst.tile([P, TQ * DC, P], F16, tag="pt", name=f"pt{t}")
                for i in range(TQ):
                    for c in range(DC):
                        nc.tensor.transpose(
                            pt[:, i * DC + c, :],
                            cx[:, i, c * P:(c + 1) * P], ident)
                cT = work.tile([P, DC, TQ, P], F16, tag="cT", name=f"cT{t}")
                nc.scalar.copy(cT, pt.rearrange("p (i c) b -> p c i b", i=TQ))

                # out_i = h_i @ Wd1 + ctx_i @ Wvd
                ob = obp.tile([P, TQ, D], F16, tag="ob", name=f"ob{t}")
                for i in range(TQ):
                    po = pso.tile([P, D], F32, tag="po", name=f"po{t}_{i}")
                    for c in range(DC):
                        nc.tensor.matmul(po, hT[:, c, i, :], wd1[:, c, :],
                                         start=(c == 0), stop=False)
                    for c in range(DC):
                        nc.tensor.matmul(po, cT[:, c, i, :], wvd[:, c, :],
                                         start=False, stop=(c == DC - 1))
                    nc.scalar.copy(ob[:, i, :], po)
                nc.sync.dma_start(out=o_r[bsl], in_=ob)

            stA, stB = {}, {}
            for tt in range(NT + 2):
                if tt < NT:
                    stA[tt] = stage_a(tt)
                if 1 <= tt < NT + 1:
                    stB[tt - 1] = stage_b(stA.pop(tt - 1))
                if tt >= 2:
                    stage_c(stB.pop(tt - 2))

    nc.compile()
    return nc


def _pack_w(w):
    # [D, D] fp32 -> [P, DC, D] fp16 with contraction dim on partitions
    return np.ascontiguousarray(
        w.reshape(DC, P, D).transpose(1, 0, 2).astype(np.float16))


def kernel(h, enc_out, Wq, Wk, Wv, Wdown, _trace=False):
    h = np.ascontiguousarray(h, dtype=np.float32)
    enc_out = np.ascontiguousarray(enc_out, dtype=np.float32)
    Wq = np.ascontiguousarray(Wq, dtype=np.float32)
    Wk = np.ascontiguousarray(Wk, dtype=np.float32)
    Wv = np.ascontiguousarray(Wv, dtype=np.float32)
    Wdown = np.ascontiguousarray(Wdown, dtype=np.float32)

    if "nc" not in _CACHED:
        _CACHED["nc"] = build()
    nc = _CACHED["nc"]

    w_a = _pack_w(Wq @ Wk.T)
    w_d1 = _pack_w(Wdown[:D])
    w_vd = _pack_w(Wv @ Wdown[D:])

    h16 = h.astype(np.float16)
    e16 = enc_out.astype(np.float16)
    e_bm = np.ascontiguousarray(e16.transpose(1, 0, 2))        # [B, TK, D]
    # block-transposed lhsT tiles: [core][t][p(d%128)][c][i][p(b%128)]
    hT_bm = np.ascontiguousarray(
        h16.reshape(TQ, NCORES, NT, P, DC, P).transpose(1, 2, 5, 4, 0, 3))
    in_maps = []
    for c in range(NCORES):
        sl = slice(c * BL, (c + 1) * BL)
        in_maps.append({
            "enc": e_bm[sl],
            "hT": hT_bm[c],
            "Wqk": w_a, "Wd1": w_d1, "Wvd": w_vd,
        })

    res = run_bass_kernel_spmd(nc, in_maps, list(range(NCORES)), trace=_trace)
    out_bm = np.concatenate([r["out"] for r in res.results], axis=0)  # [B, TQ, D]
    out = np.ascontiguousarray(out_bm.transpose(1, 0, 2))
    if _trace:
        kernel.last_result = res
    return out.astype(np.float32)
